# revision 8
# baseline (speedup 1.0000x reference)
"""Transformer encoder layer (post-norm, 16 heads, d_model=1024, d_ff=4096)
on 8 Trainium2 NeuronCores.

Sharding: batch(4) x seq-half(2) -> 8 shards; each core computes K/V for its
batch's full sequence and everything else for its 1024-query half. No
collectives.

v2: fp8(e4m3) attention path (x, Wq/Wk/Wv/Wo scaled x32, scores, exp weights,
V, concat) with DoubleRow matmuls for the QKV/O projections and attn*V;
FFN stays bf16. Fully block-pipelined: per 128-query block the kernel runs
scores -> exp(1024-wide) -> attnV -> O-proj -> LN1 -> FFN1 -> FFN2 -> LN2,
so the scalar-engine exp stream overlaps PE work of neighboring blocks.
Softmax uses exp(s/8 - 2) unnormalized weights (no max subtraction; the -2
shift keeps exp in fp8 range) -- exact after normalization.
"""

import numpy as np
import ml_dtypes

B, S, D = 4, 2048, 1024
H, DK = 16, 64
DFF = 4096
SQ = S // 2          # queries per core
P = 128              # partitions
EPS = 1e-6
NCORES = 8
WS = 32.0            # fp8 weight prescale
SHIFT = 2.0          # exp shift (cancels in softmax)

BF16 = ml_dtypes.bfloat16
FP8 = ml_dtypes.float8_e4m3

_PROG = None


def _build_program():
    import concourse.bacc as bacc
    import concourse.tile as tile
    import concourse.mybir as mybir
    from concourse.masks import make_identity

    f32 = mybir.dt.float32
    bf16 = mybir.dt.bfloat16
    fp8 = mybir.dt.float8e4
    AF = mybir.ActivationFunctionType
    Alu = mybir.AluOpType
    DR = mybir.MatmulPerfMode.DoubleRow

    nc = bacc.Bacc("TRN2", target_bir_lowering=False, debug=False,
                   num_devices=NCORES)

    # ---- DRAM parameters (per-core shards supplied by host) ----
    xt = nc.declare_dram_parameter("xt", [D, S], fp8, isOutput=False)     # x[b].T fp8
    xh = nc.declare_dram_parameter("xh", [SQ, D], bf16, isOutput=False)   # x_half + bo
    wq = nc.declare_dram_parameter("wq", [D, D], fp8, isOutput=False)     # 32*Wq
    wk = nc.declare_dram_parameter("wk", [D, D], fp8, isOutput=False)
    wv = nc.declare_dram_parameter("wv", [D, D], fp8, isOutput=False)
    wo = nc.declare_dram_parameter("wo", [D, D], fp8, isOutput=False)
    w1 = nc.declare_dram_parameter("w1", [D, DFF], bf16, isOutput=False)  # alpha1-folded
    w2 = nc.declare_dram_parameter("w2", [DFF, D], bf16, isOutput=False)
    bq = nc.declare_dram_parameter("bq", [D], f32, isOutput=False)        # 32*bq
    bk = nc.declare_dram_parameter("bk", [D], f32, isOutput=False)        # 32*bk
    bvh = nc.declare_dram_parameter("bvh", [D], bf16, isOutput=False)     # 32*bv bf16
    b1p = nc.declare_dram_parameter("b1", [DFF], f32, isOutput=False)
    a1p = nc.declare_dram_parameter("alpha1", [D], bf16, isOutput=False)
    g1p = nc.declare_dram_parameter("beta1", [D], bf16, isOutput=False)   # bias1+b2
    a2p = nc.declare_dram_parameter("alpha2", [D], bf16, isOutput=False)
    g2p = nc.declare_dram_parameter("beta2", [D], bf16, isOutput=False)
    out = nc.declare_dram_parameter("out", [SQ, D], bf16, isOutput=True)

    KC = D // P          # 8 feature chunks
    SCH = S // P         # 16 key chunks
    NBLK = SQ // P       # 8 query blocks per core

    import concourse.bass as bass

    def bcast(ap_1d, n):
        return bass.AP(tensor=ap_1d.tensor, offset=ap_1d.offset,
                       ap=[[0, P]] + list(ap_1d.ap[-1:]))[:, 0:n]

    with tile.TileContext(nc) as tc:
        with tc.tile_pool(name="main", bufs=1) as mp, \
             tc.tile_pool(name="etp", bufs=2) as etp, \
             tc.tile_pool(name="qbp", bufs=1) as qbp, \
             tc.tile_pool(name="xhp", bufs=1) as xhp, \
             tc.tile_pool(name="small", bufs=2) as smp, \
             tc.tile_pool(name="psc", bufs=2, space="PSUM") as pscp, \
             tc.tile_pool(name="pmm", bufs=2, space="PSUM") as pmm, \
             tc.tile_pool(name="psat", bufs=2, space="PSUM") as psat:

            # ---- constants ----
            ident_bf = mp.tile([P, P], bf16, tag="ident_bf")
            make_identity(nc, ident_bf)

            bq_sb = mp.tile([P, KC], f32, tag="bq")
            nc.sync.dma_start(out=bq_sb, in_=bq[:].rearrange("(c p) -> p c", p=P))
            bk_sb = mp.tile([P, KC], f32, tag="bk")
            nc.sync.dma_start(out=bk_sb, in_=bk[:].rearrange("(c p) -> p c", p=P))
            b1_sb = mp.tile([P, DFF // P], f32, tag="b1")
            nc.sync.dma_start(out=b1_sb, in_=b1p[:].rearrange("(c p) -> p c", p=P))
            bv_b = mp.tile([P, D], bf16, tag="bv_b")
            nc.sync.dma_start(out=bv_b, in_=bcast(bvh[:], D))
            g1_b = mp.tile([P, D], bf16, tag="g1_b")
            nc.sync.dma_start(out=g1_b, in_=bcast(g1p[:], D))
            a2_b = mp.tile([P, D], bf16, tag="a2_b")
            nc.sync.dma_start(out=a2_b, in_=bcast(a2p[:], D))
            g2_b = mp.tile([P, D], bf16, tag="g2_b")
            nc.sync.dma_start(out=g2_b, in_=bcast(g2p[:], D))

            nbias = mp.tile([P, 1], f32, tag="nbias")
            nc.vector.memset(nbias, -float(SHIFT))
            # prepay the exp ACT table load
            warm = mp.tile([P, 1], f32, tag="warm")
            nc.vector.memset(warm, 0.0)
            nc.scalar.activation(warm, warm, AF.Exp)

            # ---- big aliased regions ----
            # R1: xtb (qkv phase) -> w1_all (block phase)
            xtb = mp.tile([P, KC, S], fp8, tag="R1")
            nc.sync.dma_start(out=xtb,
                              in_=xt[:, :].rearrange("(c p) s -> p c s", p=P))
            # R2: wqkv (qkv phase) -> w2_all (block phase)
            wqkv = mp.tile([P, 3, KC, D], fp8, tag="R2")
            nc.sync.dma_start(out=wqkv[:, 0], in_=wq[:, :].rearrange("(c p) n -> p c n", p=P))
            nc.sync.dma_start(out=wqkv[:, 1], in_=wk[:, :].rearrange("(c p) n -> p c n", p=P))
            nc.sync.dma_start(out=wqkv[:, 2], in_=wv[:, :].rearrange("(c p) n -> p c n", p=P))
            wo_sb = mp.tile([P, KC, D], fp8, tag="wo_sb")
            nc.sync.dma_start(out=wo_sb, in_=wo[:, :].rearrange("(c p) n -> p c n", p=P))

            ktb = mp.tile([P, KC, S], fp8, tag="ktb")
            qnat = mp.tile([P, KC, SQ], fp8, tag="qnat")
            vaug = mp.tile([P, SCH, H * (DK + 1)], fp8, tag="vaug")
            va_view = vaug.rearrange("p s (h w) -> p s h w", w=DK + 1)
            nc.vector.memset(va_view[:, :, :, DK:DK + 1], 1.0)

            # ================= QKV projections (fp8 DoubleRow) =================
            with nc.named_scope("qkv"):
                # Q: feature-major [D, SQ] natural layout (head h in chunk h//2,
                # partitions (h%2)*64..)
                for dch in range(KC):
                    pts = [pmm.tile([P, 512], f32, tag="mm", name=f"ptq{i}")
                           for i in range(2)]
                    for k4 in range(4):
                        for n in range(2):
                            nc.tensor.matmul(
                                pts[n],
                                wqkv[:, 0, 2 * k4:2 * k4 + 2, dch * P:(dch + 1) * P],
                                xtb[:, 2 * k4:2 * k4 + 2, n * 512:(n + 1) * 512],
                                start=(k4 == 0), stop=(k4 == 3), perf_mode=DR)
                    for n in range(2):
                        nc.scalar.activation(
                            qnat[:, dch, n * 512:(n + 1) * 512], pts[n],
                            AF.Identity, bias=bq_sb[:, dch:dch + 1])

                # K: feature-major [D, S]
                for dch in range(KC):
                    for half in range(2):
                        pts = [pmm.tile([P, 512], f32, tag="mm", name=f"ptk{i}")
                               for i in range(2)]
                        for k4 in range(4):
                            for n in range(2):
                                nc.tensor.matmul(
                                    pts[n],
                                    wqkv[:, 1, 2 * k4:2 * k4 + 2, dch * P:(dch + 1) * P],
                                    xtb[:, 2 * k4:2 * k4 + 2,
                                        (half * 2 + n) * 512:(half * 2 + n + 1) * 512],
                                    start=(k4 == 0), stop=(k4 == 3), perf_mode=DR)
                        for n in range(2):
                            nc.scalar.activation(
                                ktb[:, dch, (half * 2 + n) * 512:(half * 2 + n + 1) * 512],
                                pts[n], AF.Identity, bias=bk_sb[:, dch:dch + 1])

                # V: token-major scattered into vaug (+32*bv), fp8
                for sch in range(SCH):
                    pts = [pmm.tile([P, 512], f32, tag="mm", name=f"ptv{i}")
                           for i in range(2)]
                    for k4 in range(4):
                        for n in range(2):
                            nc.tensor.matmul(
                                pts[n],
                                xtb[:, 2 * k4:2 * k4 + 2, sch * P:(sch + 1) * P],
                                wqkv[:, 2, 2 * k4:2 * k4 + 2, n * 512:(n + 1) * 512],
                                start=(k4 == 0), stop=(k4 == 3), perf_mode=DR)
                    for n in range(2):
                        h0 = n * (512 // DK)
                        nc.vector.tensor_add(
                            va_view[:, sch, h0:h0 + 8, 0:DK],
                            pts[n].rearrange("p (h w) -> p h w", w=DK),
                            bv_b[:, n * 512:(n + 1) * 512].rearrange(
                                "p (h w) -> p h w", w=DK))

            # alpha1 broadcast reuses bv_b's buffer (dead after V-proj)
            a1_b = mp.tile([P, D], bf16, tag="bv_b", name="a1_b")
            nc.sync.dma_start(out=a1_b, in_=bcast(a1p[:], D))

            # big FFN weights (loaded once xtb/wqkv regions drain)
            w1_all = mp.tile([P, KC, DFF], bf16, tag="R1")
            for q4 in range(4):
                nc.sync.dma_start(
                    out=w1_all[:, :, q4 * D:(q4 + 1) * D],
                    in_=w1[:, q4 * D:(q4 + 1) * D].rearrange("(c p) n -> p c n", p=P))
            w2_all = mp.tile([P, DFF // P, D], bf16, tag="R2")
            for q4 in range(4):
                nc.sync.dma_start(
                    out=w2_all[:, q4 * 8:(q4 + 1) * 8, :],
                    in_=w2[q4 * D:(q4 + 1) * D, :].rearrange("(c p) n -> p c n", p=P))

            # ================= block pipeline =================
            for j in range(NBLK):
                xh_t = xhp.tile([P, D], bf16, tag="xh", name=f"xh_{j}")
                nc.sync.dma_start(out=xh_t, in_=xh[j * P:(j + 1) * P, :])

                at2 = qbp.tile([P, H, DK], bf16, tag="at2", name=f"at2_{j}")
                with nc.named_scope(f"attn{j}"):
                    for h in range(H):
                        khc, hp0 = h // 2, (h % 2) * 64
                        et = etp.tile([P, SCH, P], fp8, tag="et", name=f"et_{j}_{h}")
                        for half in range(2):
                            psc = pscp.tile([P, 8, P], f32, tag="sc",
                                            name=f"psc_{j}_{h}_{half}")
                            for s8 in range(8):
                                sch = half * 8 + s8
                                nc.tensor.matmul(
                                    psc[:, s8, :],
                                    ktb[hp0:hp0 + 64, khc, sch * P:(sch + 1) * P],
                                    qnat[hp0:hp0 + 64, khc, j * P:(j + 1) * P],
                                    start=True, stop=True)
                            nc.scalar.activation(
                                et[:, half * 8:(half + 1) * 8, :], psc, AF.Exp,
                                bias=nbias, scale=float(1.0 / (np.sqrt(DK) * WS * WS)))
                        pat = psat.tile([P, DK + 1], f32, tag="at", name=f"pat_{j}_{h}")
                        for s4 in range(8):
                            nc.tensor.matmul(
                                pat,
                                et[:, 2 * s4:2 * s4 + 2, :],
                                vaug[:, 2 * s4:2 * s4 + 2,
                                     h * (DK + 1):(h + 1) * (DK + 1)],
                                start=(s4 == 0), stop=(s4 == 7), perf_mode=DR)
                        rec = smp.tile([P, 1], f32, tag="rec", name=f"rec_{j}_{h}")
                        nc.vector.reciprocal(rec, pat[:, DK:DK + 1])
                        nc.vector.tensor_scalar_mul(
                            at2[:, h, :], pat[:, 0:DK], rec)

                # pair-transpose -> concatT fp8 [D-chunk, q]
                concatT = qbp.tile([P, KC, P], fp8, tag="concatT",
                                   name=f"concatT_{j}")
                with nc.named_scope(f"oln{j}"):
                    for hp in range(KC):
                        ptr = psat.tile([P, P], bf16, tag="at", name=f"ptr_{j}_{hp}")
                        nc.tensor.transpose(
                            ptr, at2[:, 2 * hp:2 * hp + 2, :].rearrange("p h w -> p (h w)"),
                            ident_bf)
                        nc.vector.tensor_copy(concatT[:, hp, :], ptr)

                    # O-projection (DR): psum = 1024*attn_out
                    pos = [pmm.tile([P, 512], f32, tag="mm", name=f"po_{j}_{i}")
                           for i in range(2)]
                    for n in range(2):
                        for c4 in range(4):
                            nc.tensor.matmul(
                                pos[n],
                                concatT[:, 2 * c4:2 * c4 + 2, :],
                                wo_sb[:, 2 * c4:2 * c4 + 2, n * 512:(n + 1) * 512],
                                start=(c4 == 0), stop=(c4 == 3), perf_mode=DR)
                        # s1 = psum/1024 + (x + bo), in place in PSUM
                        nc.vector.scalar_tensor_tensor(
                            pos[n], pos[n], float(1.0 / (WS * WS)),
                            xh_t[:, n * 512:(n + 1) * 512],
                            op0=Alu.mult, op1=Alu.add)

                    # LN1 -> zb (bf16, pre-affine)
                    zb = qbp.tile([P, D], bf16, tag="zb", name=f"zb_{j}")
                    stats = smp.tile([P, 2, 6], f32, tag="stats", name=f"st1_{j}")
                    for n in range(2):
                        nc.vector.bn_stats(stats[:, n, :], pos[n])
                    mv = smp.tile([P, 2], f32, tag="mv", name=f"mv1_{j}")
                    nc.vector.bn_aggr(mv, stats)
                    std_t = smp.tile([P, 1], f32, tag="std", name=f"std1_{j}")
                    nc.scalar.activation(std_t, mv[:, 1:2], AF.Sqrt,
                                         scale=float(D / (D - 1)))
                    nc.vector.tensor_scalar_add(std_t, std_t, float(EPS))
                    rec1 = smp.tile([P, 1], f32, tag="rec1", name=f"rec1_{j}")
                    nc.vector.reciprocal(rec1, std_t)
                    mean1 = smp.tile([P, 1], f32, tag="mean", name=f"mean1_{j}")
                    nc.vector.tensor_copy(mean1, mv[:, 0:1])
                    for n in range(2):
                        nc.vector.tensor_scalar(
                            zb[:, n * 512:(n + 1) * 512], pos[n], mean1, rec1,
                            op0=Alu.subtract, op1=Alu.mult)

                    # transpose zb -> norm1T (bf16, feature-major)
                    norm1T = qbp.tile([P, KC, P], bf16, tag="norm1T",
                                      name=f"norm1T_{j}")
                    for c in range(KC):
                        ptz = psat.tile([P, P], bf16, tag="at", name=f"ptz_{j}_{c}")
                        nc.tensor.transpose(ptz, zb[:, c * P:(c + 1) * P], ident_bf)
                        nc.scalar.activation(norm1T[:, c, :], ptz, AF.Copy)

                    # affine residual in place: zb = alpha1*zb + (bias1+b2)
                    nc.gpsimd.tensor_mul(zb, zb, a1_b)
                    nc.gpsimd.tensor_add(zb, zb, g1_b)

                # FFN1: relu = max(W1.T @ norm1T + b1, 0), bf16
                relu = mp.tile([P, DFF // P, P], bf16, tag="relu", name=f"relu_{j}")
                with nc.named_scope(f"ffn1_{j}"):
                    for ch in range(DFF // P):
                        pf = pmm.tile([P, P], f32, tag="mm", name=f"pf_{j}_{ch}")
                        for kc in range(KC):
                            nc.tensor.matmul(
                                pf,
                                w1_all[:, kc, ch * P:(ch + 1) * P],
                                norm1T[:, kc, :],
                                start=(kc == 0), stop=(kc == KC - 1))
                        nc.vector.tensor_scalar(
                            relu[:, ch, :], pf, b1_sb[:, ch:ch + 1], 0.0,
                            op0=Alu.add, op1=Alu.max)

                # FFN2 + LN2 + out
                s2 = xhp.tile([P, D], bf16, tag="xh", name=f"s2_{j}")
                with nc.named_scope(f"ffn2_{j}"):
                    st2 = smp.tile([P, 2, 6], f32, tag="stats", name=f"st2_{j}")
                    for ncol in range(4):
                        pt = pmm.tile([P, 256], f32, tag="mm", name=f"pt2_{j}_{ncol}")
                        for kc in range(DFF // P):
                            nc.tensor.matmul(
                                pt,
                                relu[:, kc, :],
                                w2_all[:, kc, ncol * 256:(ncol + 1) * 256],
                                start=(kc == 0), stop=(kc == DFF // P - 1))
                        nc.vector.tensor_add(
                            s2[:, ncol * 256:(ncol + 1) * 256], pt,
                            zb[:, ncol * 256:(ncol + 1) * 256])
                        if ncol % 2 == 1:
                            nc.vector.bn_stats(
                                st2[:, ncol // 2, :],
                                s2[:, (ncol - 1) * 256:(ncol + 1) * 256])
                    mv2 = smp.tile([P, 2], f32, tag="mv", name=f"mv2_{j}")
                    nc.vector.bn_aggr(mv2, st2)
                    std2 = smp.tile([P, 1], f32, tag="std", name=f"std2_{j}")
                    nc.scalar.activation(std2, mv2[:, 1:2], AF.Sqrt,
                                         scale=float(D / (D - 1)))
                    nc.vector.tensor_scalar_add(std2, std2, float(EPS))
                    rec2 = smp.tile([P, 1], f32, tag="rec1", name=f"rec2_{j}")
                    nc.vector.reciprocal(rec2, std2)
                    mean2 = smp.tile([P, 1], f32, tag="mean", name=f"mean2_{j}")
                    nc.vector.tensor_copy(mean2, mv2[:, 0:1])
                    for n in range(2):
                        nc.vector.tensor_scalar(
                            s2[:, n * 512:(n + 1) * 512],
                            s2[:, n * 512:(n + 1) * 512], mean2, rec2,
                            op0=Alu.subtract, op1=Alu.mult)
                    # affine halves on separate engines
                    nc.vector.tensor_mul(s2[:, 0:512], s2[:, 0:512], a2_b[:, 0:512])
                    nc.gpsimd.tensor_mul(s2[:, 512:1024], s2[:, 512:1024],
                                         a2_b[:, 512:1024])
                    nc.vector.tensor_add(s2[:, 0:512], s2[:, 0:512], g2_b[:, 0:512])
                    nc.gpsimd.tensor_add(s2[:, 512:1024], s2[:, 512:1024],
                                         g2_b[:, 512:1024])
                    nc.sync.dma_start(out=out[j * P:(j + 1) * P, 0:512],
                                      in_=s2[:, 0:512])
                    nc.sync.dma_start(out=out[j * P:(j + 1) * P, 512:1024],
                                      in_=s2[:, 512:1024])

    nc.compile()
    return nc


def _get_program():
    global _PROG
    if _PROG is None:
        _PROG = _build_program()
    return _PROG


def make_in_maps(x, Wq, bq, Wk, bk, Wv, bv, Wo, bo, alpha1, bias1, alpha2,
                 bias2, W1, b1, W2, b2):
    """Build the 8 per-core input maps. Shared arrays reused by reference."""
    def f8(a, scale=1.0):
        return np.ascontiguousarray(
            np.asarray(a, np.float32) * scale).astype(FP8)

    def b16(a):
        return np.ascontiguousarray(np.asarray(a, np.float32)).astype(BF16)

    shared = {
        "wq": f8(Wq, WS), "wk": f8(Wk, WS), "wv": f8(Wv, WS), "wo": f8(Wo, WS),
        "w1": b16(np.asarray(alpha1, np.float32)[:, None]
                  * np.asarray(W1, np.float32)),
        "w2": b16(W2),
        "bq": np.asarray(bq, np.float32) * WS,
        "bk": np.asarray(bk, np.float32) * WS,
        "bvh": b16(np.asarray(bv, np.float32) * WS),
        "b1": (np.asarray(b1, np.float32)
               + np.asarray(bias1, np.float32) @ np.asarray(W1, np.float32)),
        "alpha1": b16(alpha1),
        "beta1": b16(np.asarray(bias1, np.float32) + np.asarray(b2, np.float32)),
        "alpha2": b16(alpha2),
        "beta2": b16(bias2),
    }
    x = np.asarray(x, np.float32)
    bo = np.asarray(bo, np.float32)
    in_maps = []
    for c in range(NCORES):
        b, j = c // 2, c % 2
        xb = x[b]
        # xt column order: this core's query half FIRST (cols 0:SQ)
        if j == 0:
            xt_np = xb.T
        else:
            xt_np = np.concatenate([xb[SQ:].T, xb[:SQ].T], axis=1)
        m = dict(shared)
        m["xt"] = f8(xt_np)
        m["xh"] = (xb[j * SQ:(j + 1) * SQ] + bo[None, :]).astype(BF16)
        in_maps.append(m)
    return in_maps


def kernel(**inputs):
    from concourse.bass_utils import run_bass_kernel_spmd

    nc = _get_program()
    in_maps = make_in_maps(**inputs)
    res = run_bass_kernel_spmd(nc, in_maps, core_ids=list(range(NCORES)))
    out = np.empty((B, S, D), np.float32)
    for c in range(NCORES):
        b, j = c // 2, c % 2
        out[b, j * SQ:(j + 1) * SQ, :] = res.results[c]["out"]
    return out
